# revision 5
# baseline (speedup 1.0000x reference)
"""Expert-parallel MoE FFN kernel for 8 Trainium2 NeuronCores.

Math (per expert e): out = gelu(x_e @ w1_e + b1_e) @ w2_e + b2_e
  x: [B=2, E=8, N=1024, D=1024], w1: [E, D, F=4096], b1: [E, F],
  w2: [E, F, D], b2: [E, D]  ->  out: [B, E, N, D]

Sharding: one expert per core (the e axis), outputs gathered on host.

Per-core kernel layout strategy (all matmuls in fp32r at full PE rate):
  - x_e [2048, 1024] is PE-transposed block-wise into xT [d, tok] in SBUF.
  - mm1: psum[f,tok] += w1[d,f].T @ xT[d,tok]  (f chunks of 128, tok 512)
  - ACT applies exact Gelu with per-partition bias b1[f] while moving
    PSUM -> SBUF hT [f, tok] (fp32r).
  - mm2: psum[tok,d] += hT[f,tok].T @ w2[f,d]  (accumulated over all f)
  - DVE adds broadcast b2 tile and the result stores to DRAM in the
    natural [tok, d] layout (no output transpose needed).
Weights are streamed per 512-token block (SBUF cannot hold w1+w2 in f32).
"""

import sys

for _p in ("/opt/trn_rl_repo", "/opt/pypackages"):
    if _p not in sys.path:
        sys.path.append(_p)

import numpy as np

B, E, N, D, F = 2, 8, 1024, 1024, 4096
TOK = B * N  # tokens per expert
TB = 512  # token block
NBLK = TOK // TB
nD = D // 128
nF = F // 128
nTS = TB // 128

_CACHE: dict = {}


def _build(reps: int = 1):
    import concourse.bacc as bacc
    import concourse.tile as tile
    from concourse import mybir
    from concourse.masks import make_identity

    F32 = mybir.dt.float32
    F32R = mybir.dt.float32r
    GELU = mybir.ActivationFunctionType.Gelu
    ADD = mybir.AluOpType.add

    nc = bacc.Bacc("TRN2", target_bir_lowering=False, debug=False, num_devices=8)

    x = nc.dram_tensor("x", [TOK, D], F32R, kind="ExternalInput").ap()
    w1 = nc.dram_tensor("w1", [D, F], F32R, kind="ExternalInput").ap()
    b1 = nc.dram_tensor("b1", [F], F32, kind="ExternalInput").ap()
    w2 = nc.dram_tensor("w2", [F, D], F32R, kind="ExternalInput").ap()
    b2 = nc.dram_tensor("b2", [D], F32, kind="ExternalInput").ap()
    out = nc.dram_tensor("out", [TOK, D], F32, kind="ExternalOutput").ap()

    import concourse.bass as bass

    with tile.TileContext(nc) as tc:
        with (
            tc.tile_pool(name="consts", bufs=1) as consts,
            tc.tile_pool(name="xTp", bufs=1) as xTp,
            tc.tile_pool(name="hTp", bufs=1) as hTp,
            tc.tile_pool(name="xlp", bufs=8) as xlp,
            tc.tile_pool(name="w1p", bufs=24) as w1p,
            tc.tile_pool(name="w2p", bufs=16) as w2p,
            tc.tile_pool(name="op", bufs=4) as op,
            tc.tile_pool(name="ps", bufs=8, space="PSUM") as ps,
        ):
            ident32 = consts.tile([128, 128], F32, tag="ident32")
            make_identity(nc, ident32)
            ident = consts.tile([128, 128], F32R, tag="ident")
            nc.gpsimd.tensor_copy(ident, ident32)

            b1_t = consts.tile([128, nF], F32, tag="b1")
            nc.sync.dma_start(out=b1_t, in_=b1.rearrange("(c p) -> p c", p=128))
            b2_t = consts.tile([128, D], F32, tag="b2")
            nc.gpsimd.dma_start(
                out=b2_t,
                in_=bass.AP(tensor=b2.tensor, offset=b2.offset, ap=[[0, 128], [1, D]]),
            )

            for blk in range(NBLK * reps):
                blk = blk % NBLK
                t0 = blk * TB

                # --- transpose x block into xT [d, tok] (fp32r) ---
                xT_t = xTp.tile([128, nD, TB], F32R, tag="xT")
                for dc in range(nD):
                    pt = ps.tile([128, TB], F32R, tag="ps")
                    for q in range(nTS):
                        xt = xlp.tile([128, 128], F32R, tag="xl")
                        nc.sync.dma_start(
                            out=xt,
                            in_=x[
                                t0 + q * 128 : t0 + (q + 1) * 128,
                                dc * 128 : (dc + 1) * 128,
                            ],
                        )
                        nc.tensor.transpose(pt[:, q * 128 : (q + 1) * 128], xt, ident)
                    nc.vector.tensor_copy(xT_t[:, dc, :], pt)

                # --- mm1 + gelu: hT [f, tok] (fp32r) ---
                hT_t = hTp.tile([128, nF, TB], F32R, tag="hT")
                for fg in range(nF // 4):  # f groups of 512
                    w1_ts = []
                    for dc in range(nD):
                        wt = w1p.tile([128, 512], F32R, tag="w1")
                        nc.sync.dma_start(
                            out=wt,
                            in_=w1[
                                dc * 128 : (dc + 1) * 128, fg * 512 : (fg + 1) * 512
                            ],
                        )
                        w1_ts.append(wt)
                    for fi in range(4):
                        fc = fg * 4 + fi
                        ph = ps.tile([128, TB], F32, tag="ps")
                        for dc in range(nD):
                            nc.tensor.matmul(
                                ph,
                                w1_ts[dc][:, fi * 128 : (fi + 1) * 128],
                                xT_t[:, dc, :],
                                start=(dc == 0),
                                stop=(dc == nD - 1),
                            )
                        nc.scalar.activation(
                            hT_t[:, fc, :], ph, GELU, bias=b1_t[:, fc : fc + 1],
                            scale=1.0,
                        )

                # --- mm2 + b2: out [tok, d] ---
                for dh in range(D // 512):
                    pos = [
                        ps.tile([128, 512], F32, tag="ps", name=f"po_{blk}_{dh}_{i}")
                        for i in range(nTS)
                    ]
                    for fc in range(nF):
                        wt = w2p.tile([128, 512], F32R, tag="w2")
                        nc.sync.dma_start(
                            out=wt,
                            in_=w2[
                                fc * 128 : (fc + 1) * 128, dh * 512 : (dh + 1) * 512
                            ],
                        )
                        for ts in range(nTS):
                            nc.tensor.matmul(
                                pos[ts],
                                hT_t[:, fc, ts * 128 : (ts + 1) * 128],
                                wt,
                                start=(fc == 0),
                                stop=(fc == nF - 1),
                            )
                    for ts in range(nTS):
                        ot = op.tile([128, 512], F32, tag="o")
                        nc.vector.tensor_tensor(
                            out=ot,
                            in0=pos[ts],
                            in1=b2_t[:, dh * 512 : (dh + 1) * 512],
                            op=ADD,
                        )
                        nc.sync.dma_start(
                            out=out[
                                t0 + ts * 128 : t0 + (ts + 1) * 128,
                                dh * 512 : (dh + 1) * 512,
                            ],
                            in_=ot,
                        )

    nc.compile()
    return nc


def _get_nc(reps: int = 1):
    key = f"nc{reps}"
    if key not in _CACHE:
        _CACHE[key] = _build(reps)
    return _CACHE[key]


def kernel(x, w1, b1, w2, b2):
    from concourse.bass_utils import run_bass_kernel_spmd

    x = np.asarray(x, dtype=np.float32)
    w1 = np.asarray(w1, dtype=np.float32)
    b1 = np.asarray(b1, dtype=np.float32)
    w2 = np.asarray(w2, dtype=np.float32)
    b2 = np.asarray(b2, dtype=np.float32)

    nc = _get_nc()
    in_maps = []
    for e in range(E):
        in_maps.append(
            {
                "x": np.ascontiguousarray(x[:, e]).reshape(TOK, D),
                "w1": np.ascontiguousarray(w1[e]),
                "b1": np.ascontiguousarray(b1[e]),
                "w2": np.ascontiguousarray(w2[e]),
                "b2": np.ascontiguousarray(b2[e]),
            }
        )
    res = run_bass_kernel_spmd(nc, in_maps, list(range(E)))
    out = np.empty((B, E, N, D), np.float32)
    for e in range(E):
        out[:, e] = res.results[e]["out"].reshape(B, N, D)
    return out


# revision 6
# speedup vs baseline: 2.1766x; 2.1766x over previous
"""Expert-parallel MoE FFN kernel for 8 Trainium2 NeuronCores.

Math (per expert e): out = gelu(x_e @ w1_e + b1_e) @ w2_e + b2_e
  x: [B=2, E=8, N=1024, D=1024], w1: [E, D, F=4096], b1: [E, F],
  w2: [E, F, D], b2: [E, D]  ->  out: [B, E, N, D]

Sharding: one expert per core (the e axis), outputs gathered on host.

Per-core kernel strategy (all matmuls in fp32r at full PE rate):
  - x_e [2048, 1024] is PE-transposed block-wise into xT [d, tok] in SBUF.
  - mm1: psum[f,tok] += w1[d,f].T @ xT[d,tok]  (f chunks of 128, tok 512)
  - ACT applies exact Gelu with per-partition bias b1[f] while moving
    PSUM -> SBUF hT [f, tok] (fp32r).
  - mm2: psum[tok,d] += hT[f,tok].T @ w2[f,d]  (accumulated over all f)
  - DVE adds broadcast b2 tile; result stores to DRAM in the natural
    [tok, d] layout (no output transpose needed).
Weights are streamed per 512-token block with large multi-dim DMAs
(dma_start issue rate, not HBM bandwidth, is the limiting resource).
Input loads ride the SP HWDGE ring; output stores ride the ACT ring so
next-block prefetch is never head-of-line blocked behind stores.
"""

import sys

for _p in ("/opt/trn_rl_repo", "/opt/pypackages"):
    if _p not in sys.path:
        sys.path.append(_p)

import numpy as np

B, E, N, D, F = 2, 8, 1024, 1024, 4096
TOK = B * N  # tokens per expert
TB = 512  # token block
NBLK = TOK // TB
nD = D // 128
nF = F // 128
nTS = TB // 128

_CACHE: dict = {}


def _build(reps: int = 1):
    import concourse.bacc as bacc
    import concourse.bass as bass
    import concourse.tile as tile
    from concourse import mybir
    from concourse.masks import make_identity

    F32 = mybir.dt.float32
    F32R = mybir.dt.float32r
    GELU = mybir.ActivationFunctionType.Gelu
    ADD = mybir.AluOpType.add

    nc = bacc.Bacc("TRN2", target_bir_lowering=False, debug=False, num_devices=8)

    x = nc.dram_tensor("x", [TOK, D], F32R, kind="ExternalInput").ap()
    w1 = nc.dram_tensor("w1", [D, F], F32R, kind="ExternalInput").ap()
    b1 = nc.dram_tensor("b1", [F], F32, kind="ExternalInput").ap()
    w2 = nc.dram_tensor("w2", [F, D], F32R, kind="ExternalInput").ap()
    b2 = nc.dram_tensor("b2", [D], F32, kind="ExternalInput").ap()
    out = nc.dram_tensor("out", [TOK, D], F32, kind="ExternalOutput").ap()

    # multi-dim views for coalesced DMAs
    x4 = x.rearrange("(blk q p) (dc c) -> blk q p dc c", q=nTS, p=128, c=128)
    w1_4 = w1.rearrange("(dc p) (fg f) -> dc p fg f", p=128, f=512)
    w2_4 = w2.rearrange("(fq fc p) (dh c) -> fq fc p dh c", fc=4, p=128, c=512)
    out4 = out.rearrange("(blk ts p) (dh c) -> blk ts p dh c", ts=nTS, p=128, c=512)

    with tile.TileContext(nc) as tc:
        with (
            tc.tile_pool(name="consts", bufs=1) as consts,
            tc.tile_pool(name="xTp", bufs=1) as xTp,
            tc.tile_pool(name="hTp", bufs=1) as hTp,
            tc.tile_pool(name="xlp", bufs=1) as xlp,
            tc.tile_pool(name="w1p", bufs=2) as w1p,
            tc.tile_pool(name="w2p", bufs=4) as w2p,
            tc.tile_pool(name="op", bufs=2) as op,
            tc.tile_pool(name="ps", bufs=8, space="PSUM") as ps,
        ):
            ident32 = consts.tile([128, 128], F32, tag="ident32")
            make_identity(nc, ident32)
            ident = consts.tile([128, 128], F32R, tag="ident")
            nc.gpsimd.tensor_copy(ident, ident32)

            b1_t = consts.tile([128, nF], F32, tag="b1")
            nc.sync.dma_start(out=b1_t, in_=b1.rearrange("(c p) -> p c", p=128))
            b2_t = consts.tile([128, D], F32, tag="b2")
            nc.gpsimd.dma_start(
                out=b2_t,
                in_=bass.AP(tensor=b2.tensor, offset=b2.offset, ap=[[0, 128], [1, D]]),
            )

            for blk in range(NBLK * reps):
                blk = blk % NBLK

                # --- load + transpose x block into xT [d, tok] (fp32r) ---
                xt = xlp.tile([128, nTS, nD, 128], F32R, tag="xl")
                nc.sync.dma_start(
                    out=xt, in_=x4[blk].rearrange("q p dc c -> p q dc c")
                )
                xT_t = xTp.tile([128, nD, TB], F32R, tag="xT")
                for dc in range(nD):
                    pt = ps.tile([128, TB], F32R, tag="ps")
                    for q in range(nTS):
                        nc.tensor.transpose(
                            pt[:, q * 128 : (q + 1) * 128], xt[:, q, dc, :], ident
                        )
                    nc.vector.tensor_copy(xT_t[:, dc, :], pt)

                # --- mm1 + gelu: hT [f, tok] (fp32r) ---
                hT_t = hTp.tile([128, nF, TB], F32R, tag="hT")
                for fg in range(nF // 4):  # f groups of 512
                    wt = w1p.tile([128, nD, 512], F32R, tag="w1")
                    nc.sync.dma_start(
                        out=wt, in_=w1_4[:, :, fg].rearrange("dc p f -> p dc f")
                    )
                    for fi in range(4):
                        fc = fg * 4 + fi
                        ph = ps.tile([128, TB], F32, tag="ps")
                        for dc in range(nD):
                            nc.tensor.matmul(
                                ph,
                                wt[:, dc, fi * 128 : (fi + 1) * 128],
                                xT_t[:, dc, :],
                                start=(dc == 0),
                                stop=(dc == nD - 1),
                            )
                        nc.scalar.activation(
                            hT_t[:, fc, :], ph, GELU, bias=b1_t[:, fc : fc + 1],
                            scale=1.0,
                        )

                # --- mm2 + b2: out [tok, d] ---
                for dh in range(D // 512):
                    pos = [
                        ps.tile([128, 512], F32, tag="ps", name=f"po_{blk}_{dh}_{i}")
                        for i in range(nTS)
                    ]
                    for fq in range(nF // 4):  # f chunks of 4x128
                        wt2 = w2p.tile([128, 4, 512], F32R, tag="w2")
                        nc.sync.dma_start(
                            out=wt2, in_=w2_4[fq, :, :, dh].rearrange("fc p c -> p fc c")
                        )
                        for fci in range(4):
                            fc = fq * 4 + fci
                            for ts in range(nTS):
                                nc.tensor.matmul(
                                    pos[ts],
                                    hT_t[:, fc, ts * 128 : (ts + 1) * 128],
                                    wt2[:, fci, :],
                                    start=(fc == 0),
                                    stop=(fc == nF - 1),
                                )
                    ot = op.tile([128, nTS, 512], F32, tag="o")
                    for ts in range(nTS):
                        nc.vector.tensor_tensor(
                            out=ot[:, ts, :],
                            in0=pos[ts],
                            in1=b2_t[:, dh * 512 : (dh + 1) * 512],
                            op=ADD,
                        )
                    nc.scalar.dma_start(
                        out=out4[blk, :, :, dh].rearrange("ts p c -> p ts c"), in_=ot
                    )

    nc.compile()
    return nc


def _get_nc(reps: int = 1):
    key = f"nc{reps}"
    if key not in _CACHE:
        _CACHE[key] = _build(reps)
    return _CACHE[key]


def kernel(x, w1, b1, w2, b2):
    from concourse.bass_utils import run_bass_kernel_spmd

    x = np.asarray(x, dtype=np.float32)
    w1 = np.asarray(w1, dtype=np.float32)
    b1 = np.asarray(b1, dtype=np.float32)
    w2 = np.asarray(w2, dtype=np.float32)
    b2 = np.asarray(b2, dtype=np.float32)

    nc = _get_nc()
    in_maps = []
    for e in range(E):
        in_maps.append(
            {
                "x": np.ascontiguousarray(x[:, e]).reshape(TOK, D),
                "w1": np.ascontiguousarray(w1[e]),
                "b1": np.ascontiguousarray(b1[e]),
                "w2": np.ascontiguousarray(w2[e]),
                "b2": np.ascontiguousarray(b2[e]),
            }
        )
    res = run_bass_kernel_spmd(nc, in_maps, list(range(E)))
    out = np.empty((B, E, N, D), np.float32)
    for e in range(E):
        out[:, e] = res.results[e]["out"].reshape(B, N, D)
    return out
